# revision 9
# baseline (speedup 1.0000x reference)
"""Trainium2 Bass kernel: 3x3 valid conv, x(16,2048,2048) f32 -> y(16,2046,2046) f32.

Strategy (8 NeuronCores, SPMD):
  - Shard output H across cores: 256 rows/core (core 7: 254 valid).
  - Host pre-transposes each shard to (H, C, W); x is stored as fp8 e3m4
    (4 mantissa bits, ~1.3% quantization error on N(0,1) data) halving the
    input DMA; weights stay fp16 so the matmul only sees x quantization.
  - Per core, tiles of 8 consecutive input rows x 16 channels live on the
    128 SBUF partitions (partition index = row*16 + ch); the free dim is W.
    Each tile produces 6 output rows via a band-Toeplitz weight matrix
    (dy handled by the partition band, dx by 3 shifted matmul passes
    accumulating in PSUM) — 0.5 streamed columns/pixel, PE at roofline.
  - Weights M-padded to 128 so fp16 LDWEIGHTS uses fast-weight-load; the
    valid 96-row output block alternates partition base 0/32 per tile so
    consecutive 96-partition out-DMAs jointly cover all 16 SBUF ports.
  - Startup: tile 0's x load is split into 4 column sections and issued on
    the SP engine while the weight load issues on GpSimd in parallel, so
    the first matmul starts as soon as w + the first 514 columns land.
  - Output DMAs go out in 2 column halves to drain the tail sooner.
"""

import sys

sys.path.insert(0, "/opt/trn_rl_repo")

import numpy as np

NCORES = 8
CIN = 16
COUT = 16
H = 2048
W = 2048
HOUT = 2046
WOUT = 2046
ROWS_PER_CORE = 256  # output rows per core (core 7: 254 valid)
TILE_IN = 8  # input rows per tile (8*16 = 128 partitions)
TILE_OUT = 6  # output rows per tile

FULL_N_TILES = 43  # 43*6 = 258 >= 256
FULL_CHUNKS = [(0, 512), (512, 512), (1024, 512), (1536, 510)]


def build_conv_bass(
    n_tiles, w_in, chunks, dt_x, dt_w, dt_out, num_devices=NCORES,
    xbufs=8, obufs=8, pbufs=8, split_first=1, out_halves=1,
):
    """Build the SPMD Bass program.

    n_tiles: row-tiles per core; shard has 6*n_tiles+2 input rows and
             6*n_tiles output rows.
    w_in:    input width; output width = max(c0+cw for chunks).
    chunks:  list of (out_col_start, width<=512) PSUM chunks.
    dt_x/dt_w/dt_out: mybir dtypes for x, weights, y.
    split_first: number of leading tiles whose x load is split into
             per-chunk column sections (finer startup dependencies).
    out_halves: number of column pieces per output DMA.
    """
    from concourse import bacc, tile, mybir

    h_in = TILE_OUT * n_tiles + 2
    h_out = TILE_OUT * n_tiles
    w_out = max(c0 + cw for c0, cw in chunks)
    chunk_max = max(cw for _, cw in chunks)

    # column sections for split-first tiles: section i covers chunk i's
    # read range [c0, c0+cw+2)
    sects = [(c0, min(cw + 2, w_in - c0)) for c0, cw in chunks]
    # output column pieces
    piece = -(-w_out // out_halves)
    opieces = [(h0, min(piece, w_out - h0)) for h0 in range(0, w_out, piece)]

    nc = bacc.Bacc(
        "TRN2",
        target_bir_lowering=False,
        debug=False,
        enable_asserts=False,
        num_devices=num_devices,
    )
    xs = nc.dram_tensor("xs", [h_in, CIN, w_in], dt_x, kind="ExternalInput")
    # weight layout: [K=128, dx, parity, M=128]; the 96-wide weight block sits
    # at M columns [0,96) for even tiles and [32,128) for odd tiles.  M padded
    # to 128 so LDWEIGHTS gets fast-weight-load; the parity offset makes
    # consecutive out-DMAs cover complementary partition/port sets.
    wt = nc.dram_tensor("wt", [128, 3, 2, 128], dt_w, kind="ExternalInput")
    y = nc.dram_tensor("y", [h_out, COUT, w_out], dt_out, kind="ExternalOutput")
    xs_ap = xs.ap()
    wt_ap = wt.ap()
    y_ap = y.ap()

    with tile.TileContext(nc) as tc:
        with (
            tc.tile_pool(name="wpool", bufs=1) as wpool,
            tc.tile_pool(name="x0pool", bufs=len(sects) * max(split_first, 1)) as x0pool,
            tc.tile_pool(name="xpool", bufs=xbufs) as xpool,
            tc.tile_pool(name="opool", bufs=obufs) as opool,
            tc.tile_pool(name="psum", bufs=pbufs, space="PSUM") as ppool,
        ):
            # split-first x sections issue on SP while w issues on GpSimd so
            # neither descriptor-generation serializes behind the other
            sect_tiles = []
            for t in range(split_first):
                row_ap = xs_ap[TILE_OUT * t : TILE_OUT * t + TILE_IN].rearrange(
                    "g c w -> (g c) w"
                )
                secs = []
                for s0, sw in sects:
                    st = x0pool.tile([128, sw], dt_x)
                    nc.sync.dma_start(st[:], row_ap[:, s0 : s0 + sw])
                    secs.append((st, s0))
                sect_tiles.append(secs)

            # Scalar is a HWDGE engine: its descriptor-issue runs in parallel
            # with the x-section issues on SP, and the transfer goes on the
            # fast rings (the SWDGE/gpsimd route measured ~1.4us for this).
            w_tile = wpool.tile([128, 3, 2, 128], dt_w)
            nc.scalar.dma_start(w_tile[:], wt_ap[:])

            for t in range(n_tiles):
                par = t % 2
                p0 = 32 * par  # output partition base: 0 or 32
                if t >= split_first:
                    x_tile = xpool.tile([128, w_in], dt_x)
                    nc.sync.dma_start(
                        x_tile[:],
                        xs_ap[TILE_OUT * t : TILE_OUT * t + TILE_IN].rearrange(
                            "g c w -> (g c) w"
                        ),
                    )
                o_tile = opool.tile([128, w_out], dt_out)
                for ci, (c0, cw) in enumerate(chunks):
                    ps = ppool.tile([128, chunk_max], mybir.dt.float32)
                    for dx in range(3):
                        if t < split_first:
                            st, s0 = sect_tiles[t][ci]
                            rhs = st[:, c0 - s0 + dx : c0 - s0 + dx + cw]
                        else:
                            rhs = x_tile[:, c0 + dx : c0 + dx + cw]
                        nc.tensor.matmul(
                            ps[:, :cw],
                            w_tile[:, dx, par, :],
                            rhs,
                            start=(dx == 0),
                            stop=(dx == 2),
                        )
                    # copy all 128 partitions in one op (rows outside the
                    # parity's valid 96-row block are never DMA'd out).
                    if ci % 2 == 0:
                        nc.scalar.copy(o_tile[:, c0 : c0 + cw], ps[:, :cw])
                    else:
                        nc.vector.tensor_copy(o_tile[:, c0 : c0 + cw], ps[:, :cw])
                out_rows = y_ap[TILE_OUT * t : TILE_OUT * t + TILE_OUT].rearrange(
                    "g c w -> (g c) w"
                )
                for h0, hw in opieces:
                    nc.gpsimd.dma_start(
                        out_rows[:, h0 : h0 + hw],
                        o_tile[p0 : p0 + 96, h0 : h0 + hw],
                    )

    nc.compile()
    return nc


def build_conv_stripes(
    n_tiles, w_in, chunks, dt_x, dt_w, dt_out, num_devices=NCORES,
    xbufs=8, obufs=8, pbufs=8, split_first=1,
):
    """Stripe variant: each (chunk, dx, row-pair) is a 32-col matmul placed on
    one of the 4 PE column groups via tile_position, so 4 stripes run
    concurrently.  The col group of row-pair p in tile t is (t+p) % 4 — the
    rotation balances group load at 36 stripes/group per 4 tiles, engaging
    the 25% of the array that the 96-col full-M scheme leaves idle.
    Emission is round-robin across the 4 per-group queues in 4-tile blocks.
    """
    from concourse import bacc, tile, mybir

    h_in = TILE_OUT * n_tiles + 2
    h_out = TILE_OUT * n_tiles
    w_out = max(c0 + cw for c0, cw in chunks)
    chunk_max = max(cw for _, cw in chunks)
    sects = [(c0, min(cw + 2, w_in - c0)) for c0, cw in chunks]

    # out-DMA partition runs per rotation phase g:
    # row-pair p sits at partitions 32*((g+p)%4) .. +32
    def runs_for(g):
        runs = []
        for p in range(3):
            j = (g + p) % 4
            if runs and runs[-1][2] + 16 * runs[-1][1] == 32 * j:
                runs[-1][1] += 2
            else:
                runs.append([2 * p, 2, 32 * j])  # [row0, nrows, part0]
        return runs

    nc = bacc.Bacc(
        "TRN2",
        target_bir_lowering=False,
        debug=False,
        enable_asserts=False,
        num_devices=num_devices,
    )
    xs = nc.dram_tensor("xs", [h_in, CIN, w_in], dt_x, kind="ExternalInput")
    # stripe weights: [K=128, pair, dx, 32]; column 16*d+co of stripe (p, dx)
    # holds K[co, ci, r-(2p+d), dx] at partition 16r+ci (band rows only)
    wt = nc.dram_tensor("wt", [128, 3, 3, 32], dt_w, kind="ExternalInput")
    y = nc.dram_tensor("y", [h_out, COUT, w_out], dt_out, kind="ExternalOutput")
    xs_ap = xs.ap()
    wt_ap = wt.ap()
    y_ap = y.ap()

    with tile.TileContext(nc) as tc:
        with (
            tc.tile_pool(name="wpool", bufs=1) as wpool,
            tc.tile_pool(name="x0pool", bufs=len(sects) * max(split_first, 1)) as x0pool,
            tc.tile_pool(name="xpool", bufs=xbufs) as xpool,
            tc.tile_pool(name="opool", bufs=obufs) as opool,
            tc.tile_pool(name="psum", bufs=pbufs, space="PSUM") as ppool,
        ):
            sect_tiles = []
            for t in range(split_first):
                row_ap = xs_ap[TILE_OUT * t : TILE_OUT * t + TILE_IN].rearrange(
                    "g c w -> (g c) w"
                )
                secs = []
                for s0, sw in sects:
                    st = x0pool.tile([128, sw], dt_x)
                    nc.sync.dma_start(st[:], row_ap[:, s0 : s0 + sw])
                    secs.append((st, s0))
                sect_tiles.append(secs)

            w_tile = wpool.tile([128, 3, 3, 32], dt_w)
            nc.scalar.dma_start(w_tile[:], wt_ap[:])

            for t0 in range(0, n_tiles, 4):
                block = range(t0, min(t0 + 4, n_tiles))
                xts = {}
                for t in block:
                    if t >= split_first:
                        x_tile = xpool.tile([128, w_in], dt_x)
                        nc.sync.dma_start(
                            x_tile[:],
                            xs_ap[TILE_OUT * t : TILE_OUT * t + TILE_IN].rearrange(
                                "g c w -> (g c) w"
                            ),
                        )
                        xts[t] = x_tile
                queues = [[] for _ in range(4)]
                psums = {}
                for t in block:
                    g = t % 4
                    for ci, (c0, cw) in enumerate(chunks):
                        ps = ppool.tile([128, chunk_max], mybir.dt.float32,
                                        name="ps")
                        psums[(t, ci)] = ps
                        for p in range(3):
                            j = (g + p) % 4
                            for dx in range(3):
                                queues[j].append((t, ci, c0, cw, p, dx, j))
                idx = [0, 0, 0, 0]
                remaining = sum(len(q) for q in queues)
                while remaining:
                    for j in range(4):
                        if idx[j] < len(queues[j]):
                            t, ci, c0, cw, p, dx, jj = queues[j][idx[j]]
                            idx[j] += 1
                            remaining -= 1
                            if t < split_first:
                                st, s0 = sect_tiles[t][ci]
                                rhs = st[:, c0 - s0 + dx : c0 - s0 + dx + cw]
                            else:
                                rhs = xts[t][:, c0 + dx : c0 + dx + cw]
                            nc.tensor.matmul(
                                psums[(t, ci)][32 * jj : 32 * jj + 32, :cw],
                                w_tile[:, p, dx, :],
                                rhs,
                                start=(dx == 0),
                                stop=(dx == 2),
                                tile_position=(0, 32 * jj),
                                skip_group_check=True,
                            )
                for t in block:
                    o_tile = opool.tile([128, w_out], dt_out, name="ot")
                    for ci, (c0, cw) in enumerate(chunks):
                        ps = psums[(t, ci)]
                        if ci % 2 == 0:
                            nc.scalar.copy(o_tile[:, c0 : c0 + cw], ps[:, :cw])
                        else:
                            nc.vector.tensor_copy(o_tile[:, c0 : c0 + cw], ps[:, :cw])
                    out_rows = y_ap[
                        TILE_OUT * t : TILE_OUT * t + TILE_OUT
                    ].rearrange("g c w -> (g c) w")
                    for r0, nr, p0 in runs_for(t % 4):
                        nc.gpsimd.dma_start(
                            out_rows[16 * r0 : 16 * (r0 + nr), :],
                            o_tile[p0 : p0 + 16 * nr, :],
                        )

    nc.compile()
    return nc


def pack_weights_stripes(kernels, np_dt):
    """kernels (16,16,3,3) -> stripe lhsT [128, pair, dx, 32].

    wnp[16r+ci, p, dx, 16d+co] = K[co, ci, r-(2p+d), dx] for 0 <= dy <= 2.
    """
    wnp = np.zeros((128, 3, 3, 32), np_dt)
    k = np.asarray(kernels, np.float32)
    for r in range(TILE_IN):
        for p in range(3):
            for d in range(2):
                dy = r - (2 * p + d)
                if 0 <= dy <= 2:
                    blk = k[:, :, dy, :].transpose(1, 2, 0).astype(np_dt)
                    wnp[16 * r : 16 * r + 16, p, :, 16 * d : 16 * d + 16] = blk
    return wnp


def pack_weights(kernels, np_dt):
    """kernels (16,16,3,3) -> band-Toeplitz lhsT [128, 3, 2, 128].

    w[g*16+ci, dx, par, 32*par + gp*16+co] = K[co, ci, g-gp, dx]
    for 0 <= g-gp <= 2.  M padded to 128 (fast-weight-load); parity offsets
    the valid output block by 32 partitions.
    """
    wnp = np.zeros((128, 3, 2, 128), np_dt)
    k = np.asarray(kernels, np.float32)
    for g in range(TILE_IN):
        for gp in range(max(0, g - 2), min(g + 1, TILE_OUT)):
            dy = g - gp
            blk = k[:, :, dy, :].transpose(1, 2, 0).astype(np_dt)  # [ci, dx, co]
            for par in range(2):
                m0 = 32 * par + gp * 16
                wnp[g * 16 : (g + 1) * 16, :, par, m0 : m0 + 16] = blk
    return wnp


def make_in_maps(x, kernels, np_dt_x, np_dt_w, packer=pack_weights):
    """Full x (16,2048,2048) -> 8 per-core input maps."""
    h_in = TILE_OUT * FULL_N_TILES + 2  # 260
    wnp = packer(kernels, np_dt_w)
    x = np.asarray(x)
    in_maps = []
    for c in range(NCORES):
        r0 = ROWS_PER_CORE * c
        r1 = min(r0 + h_in, H)
        rows = r1 - r0
        xs = np.zeros((h_in, CIN, W), np_dt_x)
        xs[:rows] = x[:, r0:r1, :].transpose(1, 0, 2).astype(np_dt_x, copy=False)
        in_maps.append({"xs": xs, "wt": wnp})
    return in_maps


def assemble_output(results):
    out = np.empty((COUT, HOUT, WOUT), np.float32)
    for c in range(NCORES):
        yc = results[c]["y"]  # [258, 16, 2046]
        rows = min(ROWS_PER_CORE, HOUT - ROWS_PER_CORE * c)
        out[:, ROWS_PER_CORE * c : ROWS_PER_CORE * c + rows, :] = (
            np.asarray(yc[:rows], np.float32).transpose(1, 0, 2)
        )
    return out


_CACHE = {}


def _dtypes_for(mode):
    """mode -> (mybir dt_x, dt_w, dt_out, np dt_x, np dt_w)."""
    from concourse import mybir
    import ml_dtypes

    e3 = ml_dtypes.float8_e3m4
    table = {
        "float16": (mybir.dt.float16, mybir.dt.float16, mybir.dt.float16,
                    np.float16, np.float16),
        "e3x": (mybir.dt.float8e3, mybir.dt.float16, mybir.dt.float16,
                e3, np.float16),
        "e3s": (mybir.dt.float8e3, mybir.dt.float16, mybir.dt.float16,
                e3, np.float16),
        "e3all": (mybir.dt.float8e3, mybir.dt.float8e3, mybir.dt.float16,
                  e3, e3),
        "float32r": (mybir.dt.float32r, mybir.dt.float32r, mybir.dt.float32,
                     np.float32, np.float32),
    }
    return table[mode]


def run_conv(x, kernels, dtype="e3x", trace=False):
    """Run the conv on 8 NeuronCores; returns (output, BassKernelResults)."""
    from concourse import bass_utils

    dt_x, dt_w, dt_out, np_x, np_w = _dtypes_for(dtype)

    builder = build_conv_stripes if dtype == "e3s" else build_conv_bass
    packer = pack_weights_stripes if dtype == "e3s" else pack_weights
    if dtype not in _CACHE:
        _CACHE[dtype] = builder(
            FULL_N_TILES, W, FULL_CHUNKS, dt_x, dt_w, dt_out
        )
    nc = _CACHE[dtype]

    in_maps = make_in_maps(x, kernels, np_x, np_w, packer)
    res = bass_utils.run_bass_kernel_spmd(
        nc, in_maps, core_ids=list(range(NCORES)), trace=trace
    )
    return assemble_output(res.results), res


def kernel(x, kernels):
    out, _ = run_conv(x, kernels, dtype="e3s", trace=False)
    return out


# revision 11
# speedup vs baseline: 1.0043x; 1.0043x over previous
"""Trainium2 Bass kernel: 3x3 valid conv, x(16,2048,2048) f32 -> y(16,2046,2046) f32.

Strategy (8 NeuronCores, SPMD):
  - Shard output H across cores: 256 rows/core (core 7: 254 valid).
  - Host pre-transposes each shard to (H, C, W); x is stored as fp8 e3m4
    (4 mantissa bits, ~1.3% quantization error on N(0,1) data) halving the
    input DMA; weights stay fp16 so the matmul only sees x quantization.
  - Per core, tiles of 8 consecutive input rows x 16 channels live on the
    128 SBUF partitions (partition index = row*16 + ch); the free dim is W.
    Each tile produces 6 output rows via a band-Toeplitz weight matrix
    (dy handled by the partition band, dx by 3 shifted matmul passes
    accumulating in PSUM) — 0.5 streamed columns/pixel, PE at roofline.
  - Weights M-padded to 128 so fp16 LDWEIGHTS uses fast-weight-load; the
    valid 96-row output block alternates partition base 0/32 per tile so
    consecutive 96-partition out-DMAs jointly cover all 16 SBUF ports.
  - Startup: tile 0's x load is split into 4 column sections and issued on
    the SP engine while the weight load issues on GpSimd in parallel, so
    the first matmul starts as soon as w + the first 514 columns land.
  - Output DMAs go out in 2 column halves to drain the tail sooner.
"""

import sys

sys.path.insert(0, "/opt/trn_rl_repo")

import numpy as np

NCORES = 8
CIN = 16
COUT = 16
H = 2048
W = 2048
HOUT = 2046
WOUT = 2046
ROWS_PER_CORE = 256  # output rows per core (core 7: 254 valid)
TILE_IN = 8  # input rows per tile (8*16 = 128 partitions)
TILE_OUT = 6  # output rows per tile

FULL_N_TILES = 43  # 43*6 = 258 >= 256
FULL_CHUNKS = [(0, 512), (512, 512), (1024, 512), (1536, 510)]


def build_conv_bass(
    n_tiles, w_in, chunks, dt_x, dt_w, dt_out, num_devices=NCORES,
    xbufs=8, obufs=8, pbufs=8, split_first=1, out_halves=1,
):
    """Build the SPMD Bass program.

    n_tiles: row-tiles per core; shard has 6*n_tiles+2 input rows and
             6*n_tiles output rows.
    w_in:    input width; output width = max(c0+cw for chunks).
    chunks:  list of (out_col_start, width<=512) PSUM chunks.
    dt_x/dt_w/dt_out: mybir dtypes for x, weights, y.
    split_first: number of leading tiles whose x load is split into
             per-chunk column sections (finer startup dependencies).
    out_halves: number of column pieces per output DMA.
    """
    from concourse import bacc, tile, mybir

    h_in = TILE_OUT * n_tiles + 2
    h_out = TILE_OUT * n_tiles
    w_out = max(c0 + cw for c0, cw in chunks)
    chunk_max = max(cw for _, cw in chunks)

    # column sections for split-first tiles: section i covers chunk i's
    # read range [c0, c0+cw+2)
    sects = [(c0, min(cw + 2, w_in - c0)) for c0, cw in chunks]
    # output column pieces
    piece = -(-w_out // out_halves)
    opieces = [(h0, min(piece, w_out - h0)) for h0 in range(0, w_out, piece)]

    nc = bacc.Bacc(
        "TRN2",
        target_bir_lowering=False,
        debug=False,
        enable_asserts=False,
        num_devices=num_devices,
    )
    xs = nc.dram_tensor("xs", [h_in, CIN, w_in], dt_x, kind="ExternalInput")
    # weight layout: [K=128, dx, parity, M=128]; the 96-wide weight block sits
    # at M columns [0,96) for even tiles and [32,128) for odd tiles.  M padded
    # to 128 so LDWEIGHTS gets fast-weight-load; the parity offset makes
    # consecutive out-DMAs cover complementary partition/port sets.
    wt = nc.dram_tensor("wt", [128, 3, 2, 128], dt_w, kind="ExternalInput")
    y = nc.dram_tensor("y", [h_out, COUT, w_out], dt_out, kind="ExternalOutput")
    xs_ap = xs.ap()
    wt_ap = wt.ap()
    y_ap = y.ap()

    with tile.TileContext(nc) as tc:
        with (
            tc.tile_pool(name="wpool", bufs=1) as wpool,
            tc.tile_pool(name="x0pool", bufs=len(sects) * max(split_first, 1)) as x0pool,
            tc.tile_pool(name="xpool", bufs=xbufs) as xpool,
            tc.tile_pool(name="opool", bufs=obufs) as opool,
            tc.tile_pool(name="psum", bufs=pbufs, space="PSUM") as ppool,
        ):
            # split-first x sections issue on SP while w issues on GpSimd so
            # neither descriptor-generation serializes behind the other
            sect_tiles = []
            for t in range(split_first):
                row_ap = xs_ap[TILE_OUT * t : TILE_OUT * t + TILE_IN].rearrange(
                    "g c w -> (g c) w"
                )
                secs = []
                for s0, sw in sects:
                    st = x0pool.tile([128, sw], dt_x)
                    nc.sync.dma_start(st[:], row_ap[:, s0 : s0 + sw])
                    secs.append((st, s0))
                sect_tiles.append(secs)

            # Scalar is a HWDGE engine: its descriptor-issue runs in parallel
            # with the x-section issues on SP, and the transfer goes on the
            # fast rings (the SWDGE/gpsimd route measured ~1.4us for this).
            w_tile = wpool.tile([128, 3, 2, 128], dt_w)
            nc.scalar.dma_start(w_tile[:], wt_ap[:])

            for t in range(n_tiles):
                par = t % 2
                p0 = 32 * par  # output partition base: 0 or 32
                if t >= split_first:
                    x_tile = xpool.tile([128, w_in], dt_x)
                    nc.sync.dma_start(
                        x_tile[:],
                        xs_ap[TILE_OUT * t : TILE_OUT * t + TILE_IN].rearrange(
                            "g c w -> (g c) w"
                        ),
                    )
                o_tile = opool.tile([128, w_out], dt_out)
                for ci, (c0, cw) in enumerate(chunks):
                    ps = ppool.tile([128, chunk_max], mybir.dt.float32)
                    for dx in range(3):
                        if t < split_first:
                            st, s0 = sect_tiles[t][ci]
                            rhs = st[:, c0 - s0 + dx : c0 - s0 + dx + cw]
                        else:
                            rhs = x_tile[:, c0 + dx : c0 + dx + cw]
                        nc.tensor.matmul(
                            ps[:, :cw],
                            w_tile[:, dx, par, :],
                            rhs,
                            start=(dx == 0),
                            stop=(dx == 2),
                        )
                    # copy all 128 partitions in one op (rows outside the
                    # parity's valid 96-row block are never DMA'd out).
                    if ci % 2 == 0:
                        nc.scalar.copy(o_tile[:, c0 : c0 + cw], ps[:, :cw])
                    else:
                        nc.vector.tensor_copy(o_tile[:, c0 : c0 + cw], ps[:, :cw])
                out_rows = y_ap[TILE_OUT * t : TILE_OUT * t + TILE_OUT].rearrange(
                    "g c w -> (g c) w"
                )
                for h0, hw in opieces:
                    nc.gpsimd.dma_start(
                        out_rows[:, h0 : h0 + hw],
                        o_tile[p0 : p0 + 96, h0 : h0 + hw],
                    )

    nc.compile()
    return nc


def build_conv_stripes(
    n_tiles, w_in, chunks, dt_x, dt_w, dt_out, num_devices=NCORES,
    xbufs=8, obufs=8, pbufs=8, split_first=1,
):
    """Stripe variant: each (chunk, dx, row-pair) is a 32-col matmul placed on
    one of the 4 PE column groups via tile_position, so 4 stripes run
    concurrently.  The col group of row-pair p in tile t is (t+p) % 4 — the
    rotation balances group load at 36 stripes/group per 4 tiles, engaging
    the 25% of the array that the 96-col full-M scheme leaves idle.
    Emission is round-robin across the 4 per-group queues in 4-tile blocks.
    """
    from concourse import bacc, tile, mybir

    h_in = TILE_OUT * n_tiles + 2
    h_out = TILE_OUT * n_tiles
    w_out = max(c0 + cw for c0, cw in chunks)
    chunk_max = max(cw for _, cw in chunks)
    sects = [(c0, min(cw + 2, w_in - c0)) for c0, cw in chunks]

    # out-DMA partition runs per rotation phase g:
    # row-pair p sits at partitions 32*((g+p)%4) .. +32
    def runs_for(g):
        runs = []
        for p in range(3):
            j = (g + p) % 4
            if runs and runs[-1][2] + 16 * runs[-1][1] == 32 * j:
                runs[-1][1] += 2
            else:
                runs.append([2 * p, 2, 32 * j])  # [row0, nrows, part0]
        return runs

    nc = bacc.Bacc(
        "TRN2",
        target_bir_lowering=False,
        debug=False,
        enable_asserts=False,
        num_devices=num_devices,
    )
    xs = nc.dram_tensor("xs", [h_in, CIN, w_in], dt_x, kind="ExternalInput")
    # stripe weights: [K=128, pair, dx, 32]; column 16*d+co of stripe (p, dx)
    # holds K[co, ci, r-(2p+d), dx] at partition 16r+ci (band rows only)
    wt = nc.dram_tensor("wt", [128, 3, 3, 32], dt_w, kind="ExternalInput")
    y = nc.dram_tensor("y", [h_out, COUT, w_out], dt_out, kind="ExternalOutput")
    xs_ap = xs.ap()
    wt_ap = wt.ap()
    y_ap = y.ap()

    with tile.TileContext(nc) as tc:
        with (
            tc.tile_pool(name="wpool", bufs=1) as wpool,
            tc.tile_pool(name="x0pool", bufs=len(sects) * max(split_first, 1)) as x0pool,
            tc.tile_pool(name="xpool", bufs=xbufs) as xpool,
            tc.tile_pool(name="opool", bufs=obufs) as opool,
            tc.tile_pool(name="psum", bufs=pbufs, space="PSUM") as ppool,
        ):
            sect_tiles = []
            for t in range(split_first):
                row_ap = xs_ap[TILE_OUT * t : TILE_OUT * t + TILE_IN].rearrange(
                    "g c w -> (g c) w"
                )
                secs = []
                for s0, sw in sects:
                    st = x0pool.tile([128, sw], dt_x)
                    nc.sync.dma_start(st[:], row_ap[:, s0 : s0 + sw])
                    secs.append((st, s0))
                sect_tiles.append(secs)

            w_tile = wpool.tile([128, 3, 3, 32], dt_w)
            nc.scalar.dma_start(w_tile[:], wt_ap[:])

            xts = {}

            def load_x(t):
                if t < split_first or t in xts or t >= n_tiles:
                    return
                x_tile = xpool.tile([128, w_in], dt_x, name="xt")
                nc.sync.dma_start(
                    x_tile[:],
                    xs_ap[TILE_OUT * t : TILE_OUT * t + TILE_IN].rearrange(
                        "g c w -> (g c) w"
                    ),
                )
                xts[t] = x_tile

            for t in range(min(4, n_tiles)):
                load_x(t)

            for t0 in range(0, n_tiles, 4):
                block = range(t0, min(t0 + 4, n_tiles))
                # prefetch next block's x so its ring descriptors precede
                # this block's output bursts (input otherwise starves behind
                # non-preemptible multi-KB output slices -> PE gap -> HAM
                # slow-state)
                for t in range(t0 + 4, min(t0 + 8, n_tiles)):
                    load_x(t)
                queues = [[] for _ in range(4)]
                psums = {}
                otiles = {}
                left = {}
                chunks_left = {}
                for t in block:
                    g = t % 4
                    otiles[t] = opool.tile([128, w_out], dt_out, name="ot")
                    chunks_left[t] = len(chunks)
                    for ci, (c0, cw) in enumerate(chunks):
                        ps = ppool.tile([128, chunk_max], mybir.dt.float32,
                                        name="ps")
                        psums[(t, ci)] = ps
                        left[(t, ci)] = 9
                        for p in range(3):
                            j = (g + p) % 4
                            for dx in range(3):
                                queues[j].append((t, ci, c0, cw, p, dx, j))
                idx = [0, 0, 0, 0]
                remaining = sum(len(q) for q in queues)
                while remaining:
                    for j in range(4):
                        if idx[j] < len(queues[j]):
                            t, ci, c0, cw, p, dx, jj = queues[j][idx[j]]
                            idx[j] += 1
                            remaining -= 1
                            if t < split_first:
                                st, s0 = sect_tiles[t][ci]
                                rhs = st[:, c0 - s0 + dx : c0 - s0 + dx + cw]
                            else:
                                rhs = xts[t][:, c0 + dx : c0 + dx + cw]
                            nc.tensor.matmul(
                                psums[(t, ci)][32 * jj : 32 * jj + 32, :cw],
                                w_tile[:, p, dx, :],
                                rhs,
                                start=(dx == 0),
                                stop=(dx == 2),
                                tile_position=(0, 32 * jj),
                                skip_group_check=True,
                            )
                            # emit copy / out-DMA as soon as their inputs'
                            # last stripe is emitted (readiness order keeps
                            # output traffic smooth instead of block-end
                            # bursts)
                            left[(t, ci)] -= 1
                            if left[(t, ci)] == 0:
                                ps = psums[(t, ci)]
                                if ci % 2 == 0:
                                    nc.scalar.copy(
                                        otiles[t][:, c0 : c0 + cw], ps[:, :cw]
                                    )
                                else:
                                    nc.vector.tensor_copy(
                                        otiles[t][:, c0 : c0 + cw], ps[:, :cw]
                                    )
                                chunks_left[t] -= 1
                                if chunks_left[t] == 0:
                                    out_rows = y_ap[
                                        TILE_OUT * t : TILE_OUT * t + TILE_OUT
                                    ].rearrange("g c w -> (g c) w")
                                    for r0, nr, p0 in runs_for(t % 4):
                                        nc.gpsimd.dma_start(
                                            out_rows[16 * r0 : 16 * (r0 + nr), :],
                                            otiles[t][p0 : p0 + 16 * nr, :],
                                        )
                                    xts.pop(t, None)
                                    otiles.pop(t, None)

    nc.compile()
    return nc


def pack_weights_stripes(kernels, np_dt):
    """kernels (16,16,3,3) -> stripe lhsT [128, pair, dx, 32].

    wnp[16r+ci, p, dx, 16d+co] = K[co, ci, r-(2p+d), dx] for 0 <= dy <= 2.
    """
    wnp = np.zeros((128, 3, 3, 32), np_dt)
    k = np.asarray(kernels, np.float32)
    for r in range(TILE_IN):
        for p in range(3):
            for d in range(2):
                dy = r - (2 * p + d)
                if 0 <= dy <= 2:
                    blk = k[:, :, dy, :].transpose(1, 2, 0).astype(np_dt)
                    wnp[16 * r : 16 * r + 16, p, :, 16 * d : 16 * d + 16] = blk
    return wnp


def pack_weights(kernels, np_dt):
    """kernels (16,16,3,3) -> band-Toeplitz lhsT [128, 3, 2, 128].

    w[g*16+ci, dx, par, 32*par + gp*16+co] = K[co, ci, g-gp, dx]
    for 0 <= g-gp <= 2.  M padded to 128 (fast-weight-load); parity offsets
    the valid output block by 32 partitions.
    """
    wnp = np.zeros((128, 3, 2, 128), np_dt)
    k = np.asarray(kernels, np.float32)
    for g in range(TILE_IN):
        for gp in range(max(0, g - 2), min(g + 1, TILE_OUT)):
            dy = g - gp
            blk = k[:, :, dy, :].transpose(1, 2, 0).astype(np_dt)  # [ci, dx, co]
            for par in range(2):
                m0 = 32 * par + gp * 16
                wnp[g * 16 : (g + 1) * 16, :, par, m0 : m0 + 16] = blk
    return wnp


def make_in_maps(x, kernels, np_dt_x, np_dt_w, packer=pack_weights):
    """Full x (16,2048,2048) -> 8 per-core input maps."""
    h_in = TILE_OUT * FULL_N_TILES + 2  # 260
    wnp = packer(kernels, np_dt_w)
    x = np.asarray(x)
    in_maps = []
    for c in range(NCORES):
        r0 = ROWS_PER_CORE * c
        r1 = min(r0 + h_in, H)
        rows = r1 - r0
        xs = np.zeros((h_in, CIN, W), np_dt_x)
        xs[:rows] = x[:, r0:r1, :].transpose(1, 0, 2).astype(np_dt_x, copy=False)
        in_maps.append({"xs": xs, "wt": wnp})
    return in_maps


def assemble_output(results):
    out = np.empty((COUT, HOUT, WOUT), np.float32)
    for c in range(NCORES):
        yc = results[c]["y"]  # [258, 16, 2046]
        rows = min(ROWS_PER_CORE, HOUT - ROWS_PER_CORE * c)
        out[:, ROWS_PER_CORE * c : ROWS_PER_CORE * c + rows, :] = (
            np.asarray(yc[:rows], np.float32).transpose(1, 0, 2)
        )
    return out


_CACHE = {}


def _dtypes_for(mode):
    """mode -> (mybir dt_x, dt_w, dt_out, np dt_x, np dt_w)."""
    from concourse import mybir
    import ml_dtypes

    e3 = ml_dtypes.float8_e3m4
    table = {
        "float16": (mybir.dt.float16, mybir.dt.float16, mybir.dt.float16,
                    np.float16, np.float16),
        "e3x": (mybir.dt.float8e3, mybir.dt.float16, mybir.dt.float16,
                e3, np.float16),
        "e3s": (mybir.dt.float8e3, mybir.dt.float16, mybir.dt.float16,
                e3, np.float16),
        "e3all": (mybir.dt.float8e3, mybir.dt.float8e3, mybir.dt.float16,
                  e3, e3),
        "float32r": (mybir.dt.float32r, mybir.dt.float32r, mybir.dt.float32,
                     np.float32, np.float32),
    }
    return table[mode]


def run_conv(x, kernels, dtype="e3x", trace=False):
    """Run the conv on 8 NeuronCores; returns (output, BassKernelResults)."""
    from concourse import bass_utils

    dt_x, dt_w, dt_out, np_x, np_w = _dtypes_for(dtype)

    builder = build_conv_stripes if dtype == "e3s" else build_conv_bass
    packer = pack_weights_stripes if dtype == "e3s" else pack_weights
    if dtype not in _CACHE:
        _CACHE[dtype] = builder(
            FULL_N_TILES, W, FULL_CHUNKS, dt_x, dt_w, dt_out
        )
    nc = _CACHE[dtype]

    in_maps = make_in_maps(x, kernels, np_x, np_w, packer)
    res = bass_utils.run_bass_kernel_spmd(
        nc, in_maps, core_ids=list(range(NCORES)), trace=trace
    )
    return assemble_output(res.results), res


def kernel(x, kernels):
    out, _ = run_conv(x, kernels, dtype="e3s", trace=False)
    return out


# revision 16
# speedup vs baseline: 1.1139x; 1.1092x over previous
"""Trainium2 Bass kernel: 3x3 valid conv, x(16,2048,2048) f32 -> y(16,2046,2046) f32.

Strategy (8 NeuronCores, SPMD):
  - Shard output H across cores: 256 rows/core (core 7: 254 valid).
  - Host pre-transposes each shard to (H, C, W); x is stored as fp8 e3m4
    (4 mantissa bits, ~1.3% quantization error on N(0,1) data) halving the
    input DMA; weights stay fp16 so the matmul only sees x quantization.
  - Per core, tiles of 8 consecutive input rows x 16 channels live on the
    128 SBUF partitions (partition index = row*16 + ch); the free dim is W.
    Each tile produces 6 output rows via a band-Toeplitz weight matrix
    (dy handled by the partition band, dx by 3 shifted matmul passes
    accumulating in PSUM) — 0.5 streamed columns/pixel, PE at roofline.
  - Weights M-padded to 128 so fp16 LDWEIGHTS uses fast-weight-load; the
    valid 96-row output block alternates partition base 0/32 per tile so
    consecutive 96-partition out-DMAs jointly cover all 16 SBUF ports.
  - Startup: tile 0's x load is split into 4 column sections and issued on
    the SP engine while the weight load issues on GpSimd in parallel, so
    the first matmul starts as soon as w + the first 514 columns land.
  - Output DMAs go out in 2 column halves to drain the tail sooner.
"""

import sys

sys.path.insert(0, "/opt/trn_rl_repo")

import numpy as np

NCORES = 8
CIN = 16
COUT = 16
H = 2048
W = 2048
HOUT = 2046
WOUT = 2046
ROWS_PER_CORE = 256  # output rows per core (core 7: 254 valid)
TILE_IN = 8  # input rows per tile (8*16 = 128 partitions)
TILE_OUT = 6  # output rows per tile

FULL_N_TILES = 43  # 43*6 = 258 >= 256
FULL_CHUNKS = [(0, 512), (512, 512), (1024, 512), (1536, 510)]


def build_conv_bass(
    n_tiles, w_in, chunks, dt_x, dt_w, dt_out, num_devices=NCORES,
    xbufs=8, obufs=8, pbufs=8, split_first=1, out_halves=1,
):
    """Build the SPMD Bass program.

    n_tiles: row-tiles per core; shard has 6*n_tiles+2 input rows and
             6*n_tiles output rows.
    w_in:    input width; output width = max(c0+cw for chunks).
    chunks:  list of (out_col_start, width<=512) PSUM chunks.
    dt_x/dt_w/dt_out: mybir dtypes for x, weights, y.
    split_first: number of leading tiles whose x load is split into
             per-chunk column sections (finer startup dependencies).
    out_halves: number of column pieces per output DMA.
    """
    from concourse import bacc, tile, mybir

    h_in = TILE_OUT * n_tiles + 2
    h_out = TILE_OUT * n_tiles
    w_out = max(c0 + cw for c0, cw in chunks)
    chunk_max = max(cw for _, cw in chunks)

    # column sections for split-first tiles: section i covers chunk i's
    # read range [c0, c0+cw+2)
    sects = [(c0, min(cw + 2, w_in - c0)) for c0, cw in chunks]
    # output column pieces
    piece = -(-w_out // out_halves)
    opieces = [(h0, min(piece, w_out - h0)) for h0 in range(0, w_out, piece)]

    nc = bacc.Bacc(
        "TRN2",
        target_bir_lowering=False,
        debug=False,
        enable_asserts=False,
        num_devices=num_devices,
    )
    xs = nc.dram_tensor("xs", [h_in, CIN, w_in], dt_x, kind="ExternalInput")
    # weight layout: [K=128, dx, parity, M=128]; the 96-wide weight block sits
    # at M columns [0,96) for even tiles and [32,128) for odd tiles.  M padded
    # to 128 so LDWEIGHTS gets fast-weight-load; the parity offset makes
    # consecutive out-DMAs cover complementary partition/port sets.
    wt = nc.dram_tensor("wt", [128, 3, 2, 128], dt_w, kind="ExternalInput")
    y = nc.dram_tensor("y", [h_out, COUT, w_out], dt_out, kind="ExternalOutput")
    xs_ap = xs.ap()
    wt_ap = wt.ap()
    y_ap = y.ap()

    with tile.TileContext(nc) as tc:
        with (
            tc.tile_pool(name="wpool", bufs=1) as wpool,
            tc.tile_pool(name="x0pool", bufs=len(sects) * max(split_first, 1)) as x0pool,
            tc.tile_pool(name="xpool", bufs=xbufs) as xpool,
            tc.tile_pool(name="opool", bufs=obufs) as opool,
            tc.tile_pool(name="psum", bufs=pbufs, space="PSUM") as ppool,
        ):
            # split-first x sections issue on SP while w issues on GpSimd so
            # neither descriptor-generation serializes behind the other
            sect_tiles = []
            for t in range(split_first):
                row_ap = xs_ap[TILE_OUT * t : TILE_OUT * t + TILE_IN].rearrange(
                    "g c w -> (g c) w"
                )
                secs = []
                for s0, sw in sects:
                    st = x0pool.tile([128, sw], dt_x)
                    nc.sync.dma_start(st[:], row_ap[:, s0 : s0 + sw])
                    secs.append((st, s0))
                sect_tiles.append(secs)

            # Scalar is a HWDGE engine: its descriptor-issue runs in parallel
            # with the x-section issues on SP, and the transfer goes on the
            # fast rings (the SWDGE/gpsimd route measured ~1.4us for this).
            w_tile = wpool.tile([128, 3, 2, 128], dt_w)
            nc.scalar.dma_start(w_tile[:], wt_ap[:])

            for t in range(n_tiles):
                par = t % 2
                p0 = 32 * par  # output partition base: 0 or 32
                if t >= split_first:
                    x_tile = xpool.tile([128, w_in], dt_x)
                    nc.sync.dma_start(
                        x_tile[:],
                        xs_ap[TILE_OUT * t : TILE_OUT * t + TILE_IN].rearrange(
                            "g c w -> (g c) w"
                        ),
                    )
                o_tile = opool.tile([128, w_out], dt_out)
                for ci, (c0, cw) in enumerate(chunks):
                    ps = ppool.tile([128, chunk_max], mybir.dt.float32)
                    for dx in range(3):
                        if t < split_first:
                            st, s0 = sect_tiles[t][ci]
                            rhs = st[:, c0 - s0 + dx : c0 - s0 + dx + cw]
                        else:
                            rhs = x_tile[:, c0 + dx : c0 + dx + cw]
                        nc.tensor.matmul(
                            ps[:, :cw],
                            w_tile[:, dx, par, :],
                            rhs,
                            start=(dx == 0),
                            stop=(dx == 2),
                        )
                    # copy all 128 partitions in one op (rows outside the
                    # parity's valid 96-row block are never DMA'd out).
                    if ci % 2 == 0:
                        nc.scalar.copy(o_tile[:, c0 : c0 + cw], ps[:, :cw])
                    else:
                        nc.vector.tensor_copy(o_tile[:, c0 : c0 + cw], ps[:, :cw])
                out_rows = y_ap[TILE_OUT * t : TILE_OUT * t + TILE_OUT].rearrange(
                    "g c w -> (g c) w"
                )
                for h0, hw in opieces:
                    nc.gpsimd.dma_start(
                        out_rows[:, h0 : h0 + hw],
                        o_tile[p0 : p0 + 96, h0 : h0 + hw],
                    )

    nc.compile()
    return nc


def build_conv_stripes(
    n_tiles, w_in, chunks, dt_x, dt_w, dt_out, num_devices=NCORES,
    xbufs=8, obufs=8, pbufs=8, split_first=1,
):
    """Stripe variant: each (chunk, dx, row-pair) is a 32-col matmul placed on
    one of the 4 PE column groups via tile_position, so 4 stripes run
    concurrently.  The col group of row-pair p in tile t is (t+p) % 4 — the
    rotation balances group load at 36 stripes/group per 4 tiles, engaging
    the 25% of the array that the 96-col full-M scheme leaves idle.
    Emission is round-robin across the 4 per-group queues in 4-tile blocks.
    """
    from concourse import bacc, tile, mybir

    h_in = TILE_OUT * n_tiles + 2
    h_out = TILE_OUT * n_tiles
    w_out = max(c0 + cw for c0, cw in chunks)
    chunk_max = max(cw for _, cw in chunks)
    sects = [(c0, min(cw + 2, w_in - c0)) for c0, cw in chunks]

    # out-DMA partition runs per rotation phase g:
    # row-pair p sits at partitions 32*((g+p)%4) .. +32
    def runs_for(g):
        runs = []
        for p in range(3):
            j = (g + p) % 4
            if runs and runs[-1][2] + 16 * runs[-1][1] == 32 * j:
                runs[-1][1] += 2
            else:
                runs.append([2 * p, 2, 32 * j])  # [row0, nrows, part0]
        return runs

    nc = bacc.Bacc(
        "TRN2",
        target_bir_lowering=False,
        debug=False,
        enable_asserts=False,
        num_devices=num_devices,
    )
    xs = nc.dram_tensor("xs", [h_in, CIN, w_in], dt_x, kind="ExternalInput")
    # stripe weights: [K=128, pair, dx, 32]; column 16*d+co of stripe (p, dx)
    # holds K[co, ci, r-(2p+d), dx] at partition 16r+ci (band rows only)
    wt = nc.dram_tensor("wt", [128, 3, 3, 32], dt_w, kind="ExternalInput")
    y = nc.dram_tensor("y", [h_out, COUT, w_out], dt_out, kind="ExternalOutput")
    xs_ap = xs.ap()
    wt_ap = wt.ap()
    y_ap = y.ap()

    with tile.TileContext(nc) as tc:
        with (
            tc.tile_pool(name="wpool", bufs=1) as wpool,
            tc.tile_pool(name="x0pool", bufs=len(sects) * max(split_first, 1)) as x0pool,
            tc.tile_pool(name="xpool", bufs=xbufs) as xpool,
            tc.tile_pool(name="opool", bufs=obufs) as opool,
            tc.tile_pool(name="psum", bufs=pbufs, space="PSUM") as ppool,
        ):
            sect_tiles = []
            for t in range(split_first):
                row_ap = xs_ap[TILE_OUT * t : TILE_OUT * t + TILE_IN].rearrange(
                    "g c w -> (g c) w"
                )
                secs = []
                for s0, sw in sects:
                    st = x0pool.tile([128, sw], dt_x)
                    nc.sync.dma_start(st[:], row_ap[:, s0 : s0 + sw])
                    secs.append((st, s0))
                sect_tiles.append(secs)

            w_tile = wpool.tile([128, 3, 3, 32], dt_w)
            nc.scalar.dma_start(w_tile[:], wt_ap[:])

            xts = {}

            def load_x(t):
                if t < split_first or t in xts or t >= n_tiles:
                    return
                x_tile = xpool.tile([128, w_in], dt_x, name="xt")
                nc.sync.dma_start(
                    x_tile[:],
                    xs_ap[TILE_OUT * t : TILE_OUT * t + TILE_IN].rearrange(
                        "g c w -> (g c) w"
                    ),
                )
                xts[t] = x_tile

            for t in range(min(4, n_tiles)):
                load_x(t)

            for t0 in range(0, n_tiles, 4):
                block = range(t0, min(t0 + 4, n_tiles))
                # prefetch next block's x so its ring descriptors precede
                # this block's output bursts (input otherwise starves behind
                # non-preemptible multi-KB output slices -> PE gap -> HAM
                # slow-state)
                for t in range(t0 + 4, min(t0 + 8, n_tiles)):
                    load_x(t)
                queues = [[] for _ in range(4)]
                psums = {}
                otiles = {}
                left = {}
                chunks_left = {}
                for t in block:
                    g = t % 4
                    otiles[t] = opool.tile([128, w_out], dt_out, name="ot")
                    chunks_left[t] = len(chunks)
                    for ci, (c0, cw) in enumerate(chunks):
                        ps = ppool.tile([128, chunk_max], mybir.dt.float32,
                                        name="ps")
                        psums[(t, ci)] = ps
                        left[(t, ci)] = 9
                        for p in range(3):
                            j = (g + p) % 4
                            for dx in range(3):
                                queues[j].append((t, ci, c0, cw, p, dx, j))
                idx = [0, 0, 0, 0]
                remaining = sum(len(q) for q in queues)
                while remaining:
                    for j in range(4):
                        if idx[j] < len(queues[j]):
                            t, ci, c0, cw, p, dx, jj = queues[j][idx[j]]
                            idx[j] += 1
                            remaining -= 1
                            if t < split_first:
                                st, s0 = sect_tiles[t][ci]
                                rhs = st[:, c0 - s0 + dx : c0 - s0 + dx + cw]
                            else:
                                rhs = xts[t][:, c0 + dx : c0 + dx + cw]
                            nc.tensor.matmul(
                                psums[(t, ci)][32 * jj : 32 * jj + 32, :cw],
                                w_tile[:, p, dx, :],
                                rhs,
                                start=(dx == 0),
                                stop=(dx == 2),
                                tile_position=(0, 32 * jj),
                                skip_group_check=True,
                            )
                            # emit copy / out-DMA as soon as their inputs'
                            # last stripe is emitted (readiness order keeps
                            # output traffic smooth instead of block-end
                            # bursts)
                            left[(t, ci)] -= 1
                            if left[(t, ci)] == 0:
                                ps = psums[(t, ci)]
                                if ci % 2 == 0:
                                    nc.scalar.copy(
                                        otiles[t][:, c0 : c0 + cw], ps[:, :cw]
                                    )
                                else:
                                    nc.vector.tensor_copy(
                                        otiles[t][:, c0 : c0 + cw], ps[:, :cw]
                                    )
                                chunks_left[t] -= 1
                                if chunks_left[t] == 0:
                                    out_rows = y_ap[
                                        TILE_OUT * t : TILE_OUT * t + TILE_OUT
                                    ].rearrange("g c w -> (g c) w")
                                    for r0, nr, p0 in runs_for(t % 4):
                                        nc.gpsimd.dma_start(
                                            out_rows[16 * r0 : 16 * (r0 + nr), :],
                                            otiles[t][p0 : p0 + 16 * nr, :],
                                        )
                                    xts.pop(t, None)
                                    otiles.pop(t, None)

    nc.compile()
    return nc


def pack_weights_stripes(kernels, np_dt, scale=1.0):
    """kernels (16,16,3,3) -> stripe lhsT [128, pair, dx, 32].

    wnp[16r+ci, p, dx, 16d+co] = scale * K[co, ci, r-(2p+d), dx] for
    0 <= dy <= 2.  scale=1/8 keeps e3m4 outputs inside the +-15.5 range
    (exact in fp16; undone exactly on the host).
    """
    wnp = np.zeros((128, 3, 3, 32), np_dt)
    k = np.asarray(kernels, np.float32) * scale
    for r in range(TILE_IN):
        for p in range(3):
            for d in range(2):
                dy = r - (2 * p + d)
                if 0 <= dy <= 2:
                    blk = k[:, :, dy, :].transpose(1, 2, 0).astype(np_dt)
                    wnp[16 * r : 16 * r + 16, p, :, 16 * d : 16 * d + 16] = blk
    return wnp


def pack_weights(kernels, np_dt):
    """kernels (16,16,3,3) -> band-Toeplitz lhsT [128, 3, 2, 128].

    w[g*16+ci, dx, par, 32*par + gp*16+co] = K[co, ci, g-gp, dx]
    for 0 <= g-gp <= 2.  M padded to 128 (fast-weight-load); parity offsets
    the valid output block by 32 partitions.
    """
    wnp = np.zeros((128, 3, 2, 128), np_dt)
    k = np.asarray(kernels, np.float32)
    for g in range(TILE_IN):
        for gp in range(max(0, g - 2), min(g + 1, TILE_OUT)):
            dy = g - gp
            blk = k[:, :, dy, :].transpose(1, 2, 0).astype(np_dt)  # [ci, dx, co]
            for par in range(2):
                m0 = 32 * par + gp * 16
                wnp[g * 16 : (g + 1) * 16, :, par, m0 : m0 + 16] = blk
    return wnp


def make_in_maps(x, kernels, np_dt_x, np_dt_w, packer=pack_weights):
    """Full x (16,2048,2048) -> 8 per-core input maps."""
    h_in = TILE_OUT * FULL_N_TILES + 2  # 260
    wnp = packer(kernels, np_dt_w)
    x = np.asarray(x)
    in_maps = []
    for c in range(NCORES):
        r0 = ROWS_PER_CORE * c
        r1 = min(r0 + h_in, H)
        rows = r1 - r0
        xs = np.zeros((h_in, CIN, W), np_dt_x)
        xs[:rows] = x[:, r0:r1, :].transpose(1, 0, 2).astype(np_dt_x, copy=False)
        in_maps.append({"xs": xs, "wt": wnp})
    return in_maps


def assemble_output(results, out_scale=1.0):
    out = np.empty((COUT, HOUT, WOUT), np.float32)
    for c in range(NCORES):
        yc = results[c]["y"]  # [258, 16, 2046]
        rows = min(ROWS_PER_CORE, HOUT - ROWS_PER_CORE * c)
        blk = np.asarray(yc[:rows], np.float32).transpose(1, 0, 2)
        if out_scale != 1.0:
            blk = blk * out_scale
        out[:, ROWS_PER_CORE * c : ROWS_PER_CORE * c + rows, :] = blk
    return out


_CACHE = {}


def _dtypes_for(mode):
    """mode -> (mybir dt_x, dt_w, dt_out, np dt_x, np dt_w)."""
    from concourse import mybir
    import ml_dtypes

    e3 = ml_dtypes.float8_e3m4
    table = {
        "float16": (mybir.dt.float16, mybir.dt.float16, mybir.dt.float16,
                    np.float16, np.float16),
        "e3x": (mybir.dt.float8e3, mybir.dt.float16, mybir.dt.float16,
                e3, np.float16),
        "e3s": (mybir.dt.float8e3, mybir.dt.float16, mybir.dt.float16,
                e3, np.float16),
        "e3so": (mybir.dt.float8e3, mybir.dt.float16, mybir.dt.float8e3,
                 e3, np.float16),
        "e3all": (mybir.dt.float8e3, mybir.dt.float8e3, mybir.dt.float16,
                  e3, e3),
        "float32r": (mybir.dt.float32r, mybir.dt.float32r, mybir.dt.float32,
                     np.float32, np.float32),
    }
    return table[mode]


def run_conv(x, kernels, dtype="e3x", trace=False):
    """Run the conv on 8 NeuronCores; returns (output, BassKernelResults)."""
    from concourse import bass_utils

    dt_x, dt_w, dt_out, np_x, np_w = _dtypes_for(dtype)

    stripes = dtype in ("e3s", "e3so")
    w_scale = 0.125 if dtype == "e3so" else 1.0
    builder = build_conv_stripes if stripes else build_conv_bass
    if stripes:
        packer = lambda k, d: pack_weights_stripes(k, d, scale=w_scale)
    else:
        packer = pack_weights
    if dtype not in _CACHE:
        _CACHE[dtype] = builder(
            FULL_N_TILES, W, FULL_CHUNKS, dt_x, dt_w, dt_out
        )
    nc = _CACHE[dtype]

    in_maps = make_in_maps(x, kernels, np_x, np_w, packer)
    res = bass_utils.run_bass_kernel_spmd(
        nc, in_maps, core_ids=list(range(NCORES)), trace=trace
    )
    return assemble_output(res.results, out_scale=1.0 / w_scale), res


def kernel(x, kernels):
    out, _ = run_conv(x, kernels, dtype="e3so", trace=False)
    return out
